# revision 5
# baseline (speedup 1.0000x reference)
"""Trainium2 Bass kernel for nn_CrossAttentionModel (8 NeuronCores).

Strategy: the only large tensors are the 4x4-downsampled activations
(a: 128x3x20480, v: 128x20480, 42 MB) and the encoder weights
W1/W2 (128x20480 each, 21 MB).  We shard the CONTRACTION dim d=20480
across the 8 cores (d-parallel): each core streams its 2560-row slice
of both the activations (all 128 samples) and the weights, accumulates
partial encoder outputs aud/vis for every sample, and a ReduceScatter
(sum over d-shards, scatter over samples) hands each core the exact
encoder outputs for its 16 samples.  This reads every weight byte and
every activation byte exactly once -> minimal HBM traffic
(63 MB / 8 cores ~ 7.9 MB/core).  The small attention head then runs
batch-parallel (16 samples/core) on-chip.

Host-side prep only does the nearest-downsample slicing + layout
transposes (sharding); all FLOPs run on device.
"""
import sys
sys.path.insert(0, "/opt/trn_rl_repo")

import numpy as np
import concourse.bass as bass
import concourse.mybir as mybir
import concourse.tile as tile
from concourse import bacc
from concourse.bass_utils import run_bass_kernel_spmd

F32 = mybir.dt.float32

# ---- problem constants (hardcoded; kernel.py must be self-contained) ----
B, C, H, W = 128, 3, 512, 640
D = 20480            # (H//4) * (W//4)
DE = 128             # encoder dim
DA = 32              # attention dim
NC_ = 8              # cores
DL = D // NC_        # 2560 d-rows per core
NT = DL // 128       # 20 k-tiles per core
SL = B // NC_        # 16 samples per core (post reduce-scatter)
NQ = 4               # sample quads per core (chunks of 512 = 4*128 cols)

# encoder matmul dtype: float32r = fp32 bits, fast PE path
EDT = mybir.dt.bfloat16
# attention compute dtype
CDT = mybir.dt.bfloat16


def _np_dt(dt):
    return mybir.dt.np(dt)


def build_bass():
    nc = bacc.Bacc("TRN2", target_bir_lowering=False, debug=False,
                   num_devices=NC_)

    # ---- per-core DRAM parameters (shards supplied host-side) ----
    aT = nc.declare_dram_parameter("aT", [128, NT * 3 * B], EDT, isOutput=False)
    vT = nc.declare_dram_parameter("vT", [128, NT * B], EDT, isOutput=False)
    w12 = nc.declare_dram_parameter("w12", [128, NT * 2 * DE], EDT,
                                    isOutput=False)
    brow = nc.declare_dram_parameter("brow", [1, 2 * DE], EDT, isOutput=False)
    ones = nc.declare_dram_parameter("ones", [1, 3 * B], EDT, isOutput=False)
    aaT3 = nc.declare_dram_parameter("aaT3", [3, 3], CDT, isOutput=False)
    avT3 = nc.declare_dram_parameter("avT3", [3, 3], CDT, isOutput=False)
    wa3 = nc.declare_dram_parameter("wa3", [3, DA], CDT, isOutput=False)
    wv3 = nc.declare_dram_parameter("wv3", [3, DA], CDT, isOutput=False)
    wcaT = nc.declare_dram_parameter("wcaT", [2 * DE, DA], CDT, isOutput=False)
    wcvT = nc.declare_dram_parameter("wcvT", [2 * DE, DA], CDT, isOutput=False)
    whaT = nc.declare_dram_parameter("whaT", [DA, 3], CDT, isOutput=False)
    whvT = nc.declare_dram_parameter("whvT", [DA, 3], CDT, isOutput=False)
    out = nc.declare_dram_parameter("out", [SL, 3, 2 * DE], F32, isOutput=True)

    # internal DRAM for the collective
    P = nc.dram_tensor("P", [B, 4, DE], F32)
    P_rs = nc.dram_tensor("P_rs", [SL, 4, DE], F32)

    SK = SL * DE  # 2048 free columns in (sample, enc) layout

    with tile.TileContext(nc) as tc:
        with (
            tc.tile_pool(name="consts", bufs=1) as cpool,
            tc.tile_pool(name="enc_in", bufs=3) as epool,
            tc.tile_pool(name="sb", bufs=1) as sb,
        ):
            # ---------- small consts ----------
            brow_t = cpool.tile([1, 2 * DE], EDT)
            nc.gpsimd.dma_start(brow_t[:], brow[:])
            ones_t = cpool.tile([1, 3 * B], EDT)
            nc.gpsimd.dma_start(ones_t[:], ones[:])
            aa_t = cpool.tile([3, 3], CDT)
            nc.gpsimd.dma_start(aa_t[:], aaT3[:])
            av_t = cpool.tile([3, 3], CDT)
            nc.gpsimd.dma_start(av_t[:], avT3[:])
            wa3_t = cpool.tile([3, DA], CDT)
            nc.gpsimd.dma_start(wa3_t[:], wa3[:])
            wv3_t = cpool.tile([3, DA], CDT)
            nc.gpsimd.dma_start(wv3_t[:], wv3[:])
            wca_lo = cpool.tile([DE, DA], CDT)
            nc.gpsimd.dma_start(wca_lo[:], wcaT[0:DE, :])
            wca_hi = cpool.tile([DE, DA], CDT)
            nc.gpsimd.dma_start(wca_hi[:], wcaT[DE:2 * DE, :])
            wcv_lo = cpool.tile([DE, DA], CDT)
            nc.gpsimd.dma_start(wcv_lo[:], wcvT[0:DE, :])
            wcv_hi = cpool.tile([DE, DA], CDT)
            nc.gpsimd.dma_start(wcv_hi[:], wcvT[DE:2 * DE, :])
            wha_t = cpool.tile([DA, 3], CDT)
            nc.gpsimd.dma_start(wha_t[:], whaT[:])
            whv_t = cpool.tile([DA, 3], CDT)
            nc.gpsimd.dma_start(whv_t[:], whvT[:])

            # ---------- phase 1: encoder (d-sharded, all 128 samples) ----------
            # psum_c[s, 0:DE] accumulates (a_c @ W1^T)[s, :] for channel c;
            # psum_v[s, DE:2DE] accumulates (v @ W2^T)[s, :].
            with tc.tile_pool(name="enc_ps", bufs=1, space="PSUM") as eps:
                psums = [eps.tile([B, 2 * DE], F32, tag=f"enc{c}",
                                  name=f"psum_enc{c}") for c in range(4)]
                # bias init (start=True clears the bank): out[s, k] = b/8
                for c in range(3):
                    nc.tensor.matmul(psums[c][:], ones_t[0:1, 0:B],
                                     brow_t[:], start=True, stop=False)
                nc.tensor.matmul(psums[3][:], ones_t[0:1, 0:B],
                                 brow_t[:], start=True, stop=False)

                TPC = NT // 4
                ats, vts, wts = [], [], []
                for ch in range(4):
                    at3 = epool.tile([128, TPC * 3 * B], EDT, tag=f"at{ch}",
                                     name=f"at{ch}")
                    nc.gpsimd.dma_start(
                        at3[:], aT[:, ch * TPC * 3 * B:(ch + 1) * TPC * 3 * B])
                    ats.append(at3)
                    vt = epool.tile([128, TPC * B], EDT, tag=f"vt{ch}",
                                    name=f"vt{ch}")
                    nc.gpsimd.dma_start(
                        vt[:], vT[:, ch * TPC * B:(ch + 1) * TPC * B])
                    vts.append(vt)
                    w12t = epool.tile([128, TPC * 2 * DE], EDT, tag=f"wt{ch}",
                                      name=f"wt{ch}")
                    nc.gpsimd.dma_start(
                        w12t[:], w12[:, ch * TPC * 2 * DE:(ch + 1) * TPC * 2 * DE])
                    wts.append(w12t)
                for ch in range(4):
                    for tt in range(TPC):
                        t = ch * TPC + tt
                        last = t == NT - 1
                        at3 = ats[ch][:, tt * 3 * B:(tt + 1) * 3 * B]
                        vt = vts[ch][:, tt * B:(tt + 1) * B]
                        w12t = wts[ch][:, tt * 2 * DE:(tt + 1) * 2 * DE]
                        for c in range(3):
                            nc.tensor.matmul(psums[c][:],
                                             at3[:, c * B:(c + 1) * B], w12t[:],
                                             start=False, stop=last)
                        nc.tensor.matmul(psums[3][:], vt[:], w12t[:],
                                         start=False, stop=last)

                # evict to P: channels use cols 0:DE (W1 half), vis DE:2DE
                for c in range(3):
                    ev = sb.tile([B, DE], F32, tag=f"ev{c}", name=f"ev{c}")
                    nc.any.tensor_copy(ev[:], psums[c][:, 0:DE])
                    nc.gpsimd.dma_start(P[:, c, :], ev[:])
                ev = sb.tile([B, DE], F32, tag="ev3", name="ev3")
                nc.any.tensor_copy(ev[:], psums[3][:, DE:2 * DE])
                nc.gpsimd.dma_start(P[:, 3, :], ev[:])

            # ---------- reduce-scatter: sum over d-shards, scatter samples ----
            nc.gpsimd.collective_compute(
                "ReduceScatter", mybir.AluOpType.add,
                replica_groups=[list(range(NC_))],
                ins=[P[:]], outs=[P_rs[:]],
            )

            # ---------- load this core's 16 samples: [ch, (s, k)] layout ----
            # aud channels and vis kept in separate partition-0-based tiles
            # (compute engines are lane-locked; no partition shifts allowed)
            av_a = sb.tile([3, SK], F32, tag="av_a")   # aud: enc1 + b1
            av_v = sb.tile([3, SK], F32, tag="av_v")   # vis (3 equal rows)
            nc.gpsimd.dma_start(
                av_a[:].rearrange("c (s k) -> c s k", k=DE),
                P_rs[:, 0:3, :].transpose([1, 0, 2]))
            for r in range(3):
                nc.gpsimd.dma_start(
                    av_v[r:r + 1, :].rearrange("c (s k) -> c s k", k=DE),
                    P_rs[:, 3:4, :].transpose([1, 0, 2]))
            av16_a = sb.tile([3, SK], CDT, tag="av16_a")
            nc.any.tensor_copy(av16_a[:], av_a[:])
            av16_v = sb.tile([3, SK], CDT, tag="av16_v")
            nc.any.tensor_copy(av16_v[:], av_v[:])

            with (
                tc.tile_pool(name="att_ps", bufs=2, space="PSUM") as aps,
                tc.tile_pool(name="h_ps", bufs=1, space="PSUM") as hps,
                tc.tile_pool(name="b_ps", bufs=1, space="PSUM") as bps,
                tc.tile_pool(name="o_ps", bufs=1, space="PSUM") as ops_,
            ):
                # ---------- B = A @ av: four K=3 products [3, SK] ----------
                # (Aa|Av) x (aud-half | vis-half) of av
                ba_lo = sb.tile([3, SK], CDT, tag="ba_lo")
                ba_hi = sb.tile([3, SK], CDT, tag="ba_hi")
                bv_lo = sb.tile([3, SK], CDT, tag="bv_lo")
                bv_hi = sb.tile([3, SK], CDT, tag="bv_hi")
                bspec = [(ba_lo, aa_t, av16_a), (ba_hi, aa_t, av16_v),
                         (bv_lo, av_t, av16_a), (bv_hi, av_t, av16_v)]
                for q in range(NQ):
                    ck = slice(q * 512, (q + 1) * 512)
                    for dst, lhs_c, rhs_c in bspec:
                        pb = bps.tile([3, 512], F32, tag="pb")
                        nc.tensor.matmul(pb[:], lhs_c[:], rhs_c[:, ck],
                                         start=True, stop=True)
                        nc.any.tensor_copy(dst[:, ck], pb[:])

                # ---------- attention maps: att = tanh((enc^T @ B) / 16) -------
                att = {
                    (br, half): sb.tile([DE, SK], CDT, tag=f"att_{br}_{half}",
                                        name=f"att_{br}_{half}")
                    for br in ("a", "v") for half in (0, 1)
                }
                blos = {"a": (ba_lo, ba_hi), "v": (bv_lo, bv_hi)}
                enc_rhs = {"a": av16_a, "v": av16_v}
                for q in range(NQ):
                    for br in ("a", "v"):
                        rhs_t = enc_rhs[br]
                        for half in (0, 1):
                            blk = blos[br][half]
                            pa = aps.tile([DE, 512], F32, tag="attp")
                            for j in range(4):
                                s = q * 4 + j
                                sl_ = slice(s * DE, (s + 1) * DE)
                                nc.tensor.matmul(
                                    pa[:, j * DE:(j + 1) * DE],
                                    blk[:, sl_], rhs_t[:, sl_],
                                    start=True, stop=True)
                            nc.scalar.activation(
                                att[(br, half)][:, q * 512:(q + 1) * 512], pa[:],
                                mybir.ActivationFunctionType.Tanh, scale=0.0625)

                # ---------- H = relu(att @ Wc^T + enc^T @ W) ----------
                ht_a = sb.tile([DA, SK], CDT, tag="ht_a")
                ht_v = sb.tile([DA, SK], CDT, tag="ht_v")
                for q in range(NQ):
                    ck = slice(q * 512, (q + 1) * 512)
                    ph_a = hps.tile([DA, 512], F32, tag="ph_a")
                    nc.tensor.matmul(ph_a[:], wa3_t[:], av16_a[:, ck],
                                     start=True, stop=False)
                    nc.tensor.matmul(ph_a[:], wca_lo[:], att[("a", 0)][:, ck],
                                     start=False, stop=False)
                    nc.tensor.matmul(ph_a[:], wca_hi[:], att[("a", 1)][:, ck],
                                     start=False, stop=True)
                    nc.scalar.activation(ht_a[:, ck], ph_a[:],
                                         mybir.ActivationFunctionType.Relu)
                    ph_v = hps.tile([DA, 512], F32, tag="ph_v")
                    nc.tensor.matmul(ph_v[:], wv3_t[:], av16_v[:, ck],
                                     start=True, stop=False)
                    nc.tensor.matmul(ph_v[:], wcv_lo[:], att[("v", 0)][:, ck],
                                     start=False, stop=False)
                    nc.tensor.matmul(ph_v[:], wcv_hi[:], att[("v", 1)][:, ck],
                                     start=False, stop=True)
                    nc.scalar.activation(ht_v[:, ck], ph_v[:],
                                         mybir.ActivationFunctionType.Relu)

                # ---------- out = Wh @ H^T + enc ----------
                outa = sb.tile([3, SK], F32, tag="outa")
                outv = sb.tile([3, SK], F32, tag="outv")
                for q in range(NQ):
                    ck = slice(q * 512, (q + 1) * 512)
                    poa = ops_.tile([3, 512], F32, tag="poa")
                    nc.tensor.matmul(poa[:], wha_t[:], ht_a[:, ck],
                                     start=True, stop=True)
                    nc.vector.tensor_add(outa[:, ck], poa[:], av_a[:, ck])
                    pov = ops_.tile([3, 512], F32, tag="pov")
                    nc.tensor.matmul(pov[:], whv_t[:], ht_v[:, ck],
                                     start=True, stop=True)
                    nc.vector.tensor_add(outv[:, ck], pov[:], av_v[:, ck])

            nc.gpsimd.dma_start(
                out[:, :, 0:DE].transpose([1, 0, 2]),
                outa[:].rearrange("c (s k) -> c s k", k=DE))
            nc.gpsimd.dma_start(
                out[:, :, DE:2 * DE].transpose([1, 0, 2]),
                outv[:].rearrange("c (s k) -> c s k", k=DE))

    nc.compile()
    return nc


_NC_CACHE = None


def _get_nc():
    global _NC_CACHE
    if _NC_CACHE is None:
        _NC_CACHE = build_bass()
    return _NC_CACHE


def _prep_inputs(f1_norm, f2_norm, W1, b1, W2, b2, Aa, Av, Wa, Wv,
                 Wca, Wcv, Wha, Whv):
    f1_norm = np.asarray(f1_norm, dtype=np.float32)
    f2_norm = np.asarray(f2_norm, dtype=np.float32)
    edt = _np_dt(EDT)
    cdt = _np_dt(CDT)

    # nearest-downsample + transpose to [d, (c, s)] / [d, s]
    a_ds = f1_norm[:, :, ::4, ::4].reshape(B, 3, D)       # (B, 3, D)
    aT_full = np.ascontiguousarray(a_ds.transpose(2, 1, 0)
                                   .reshape(D, 3 * B)).astype(edt, copy=False)
    v_ds = f2_norm[:, ::4, ::4].reshape(B, D)
    vT_full = np.ascontiguousarray(v_ds.T).astype(edt, copy=False)
    w12_full = np.ascontiguousarray(
        np.concatenate([np.asarray(W1).T, np.asarray(W2).T], axis=1)
    ).astype(edt, copy=False)                              # (D, 256)

    brow = np.concatenate([np.asarray(b1), np.asarray(b2)])[None, :] / NC_
    brow = brow.astype(edt)
    ones = np.ones((1, 3 * B), dtype=edt)

    consts = {
        "brow": brow, "ones": ones,
        "aaT3": np.ascontiguousarray(np.asarray(Aa).T).astype(cdt),
        "avT3": np.ascontiguousarray(np.asarray(Av).T).astype(cdt),
        "wa3": np.ascontiguousarray(np.asarray(Wa).T).astype(cdt),
        "wv3": np.ascontiguousarray(np.asarray(Wv).T).astype(cdt),
        "wcaT": np.ascontiguousarray(np.asarray(Wca).T).astype(cdt),
        "wcvT": np.ascontiguousarray(np.asarray(Wcv).T).astype(cdt),
        "whaT": np.ascontiguousarray(np.asarray(Wha).T).astype(cdt),
        "whvT": np.ascontiguousarray(np.asarray(Whv).T).astype(cdt),
    }

    def tile128(arr, ncols):
        # [DL, ncols] -> [128, NT*ncols]: row p holds k-tiles t at col t*ncols
        return np.ascontiguousarray(
            arr.reshape(NT, 128, ncols).transpose(1, 0, 2)
            .reshape(128, NT * ncols))

    in_maps = []
    for i in range(NC_):
        rs = slice(i * DL, (i + 1) * DL)
        m = {
            "aT": tile128(aT_full[rs], 3 * B),
            "vT": tile128(vT_full[rs], B),
            "w12": tile128(w12_full[rs], 2 * DE),
        }
        m.update(consts)
        in_maps.append(m)
    return in_maps


def _run(inputs, trace=False):
    nc = _get_nc()
    in_maps = _prep_inputs(**inputs)
    res = run_bass_kernel_spmd(nc, in_maps, list(range(NC_)), trace=trace)
    full = np.concatenate([res.results[i]["out"] for i in range(NC_)], axis=0)
    return full.astype(np.float32, copy=False), res


def kernel(**inputs):
    out, _ = _run(inputs, trace=False)
    return out



# revision 6
# speedup vs baseline: 1.0634x; 1.0634x over previous
"""Trainium2 Bass kernel for nn_CrossAttentionModel (8 NeuronCores).

Strategy: the only large tensors are the 4x4-downsampled activations
(a: 128x3x20480, v: 128x20480, 42 MB) and the encoder weights
W1/W2 (128x20480 each, 21 MB).  We shard the CONTRACTION dim d=20480
across the 8 cores (d-parallel): each core streams its 2560-row slice
of both the activations (all 128 samples) and the weights, accumulates
partial encoder outputs aud/vis for every sample, and a ReduceScatter
(sum over d-shards, scatter over samples) hands each core the exact
encoder outputs for its 16 samples.  This reads every weight byte and
every activation byte exactly once -> minimal HBM traffic
(63 MB / 8 cores ~ 7.9 MB/core).  The small attention head then runs
batch-parallel (16 samples/core) on-chip.

Host-side prep only does the nearest-downsample slicing + layout
transposes (sharding); all FLOPs run on device.
"""
import sys
sys.path.insert(0, "/opt/trn_rl_repo")

import numpy as np
import concourse.bass as bass
import concourse.mybir as mybir
import concourse.tile as tile
from concourse import bacc
from concourse.bass_utils import run_bass_kernel_spmd

F32 = mybir.dt.float32

# ---- problem constants (hardcoded; kernel.py must be self-contained) ----
B, C, H, W = 128, 3, 512, 640
D = 20480            # (H//4) * (W//4)
DE = 128             # encoder dim
DA = 32              # attention dim
NC_ = 8              # cores
DL = D // NC_        # 2560 d-rows per core
NT = DL // 128       # 20 k-tiles per core
SL = B // NC_        # 16 samples per core (post reduce-scatter)
NQ = 4               # sample quads per core (chunks of 512 = 4*128 cols)

# encoder matmul dtype: float32r = fp32 bits, fast PE path
EDT = mybir.dt.bfloat16
# attention compute dtype
CDT = mybir.dt.bfloat16


def _np_dt(dt):
    return mybir.dt.np(dt)


def build_bass():
    nc = bacc.Bacc("TRN2", target_bir_lowering=False, debug=False,
                   num_devices=NC_)

    # ---- per-core DRAM parameters (shards supplied host-side) ----
    aT = nc.declare_dram_parameter("aT", [128, NT * 3 * B], EDT, isOutput=False)
    vT = nc.declare_dram_parameter("vT", [128, NT * B], EDT, isOutput=False)
    w12 = nc.declare_dram_parameter("w12", [128, NT * 2 * DE], EDT,
                                    isOutput=False)
    pk16 = nc.declare_dram_parameter("pk16", [128, 844], CDT, isOutput=False)
    out = nc.declare_dram_parameter("out", [SL, 3, 2 * DE], F32, isOutput=True)

    # internal DRAM for the collective
    P = nc.dram_tensor("P", [B, 4, DE], F32)
    P_rs = nc.dram_tensor("P_rs", [SL, 4, DE], F32)

    SK = SL * DE  # 2048 free columns in (sample, enc) layout

    with tile.TileContext(nc) as tc:
        with (
            tc.tile_pool(name="consts", bufs=1) as cpool,
            tc.tile_pool(name="enc_in", bufs=3) as epool,
            tc.tile_pool(name="sb", bufs=1) as sb,
        ):
            # ---------- small consts ----------
            pkc = cpool.tile([128, 844], CDT)
            nc.gpsimd.dma_start(pkc[:], pk16[:])
            wca_lo = pkc[:, 0:DA]
            wca_hi = pkc[:, DA:2 * DA]
            wcv_lo = pkc[:, 2 * DA:3 * DA]
            wcv_hi = pkc[:, 3 * DA:4 * DA]
            aa_t = pkc[0:3, 128:131]
            av_t = pkc[0:3, 131:134]
            wa3_t = pkc[0:3, 134:166]
            wv3_t = pkc[0:3, 166:198]
            wha_t = pkc[0:DA, 198:201]
            whv_t = pkc[0:DA, 201:204]
            brow_t = pkc[0:1, 204:460]
            ones_t = pkc[0:1, 460:844]

            # ---------- phase 1: encoder (d-sharded, all 128 samples) ----------
            # psum_c[s, 0:DE] accumulates (a_c @ W1^T)[s, :] for channel c;
            # psum_v[s, DE:2DE] accumulates (v @ W2^T)[s, :].
            with tc.tile_pool(name="enc_ps", bufs=1, space="PSUM") as eps:
                psums = [eps.tile([B, 2 * DE], F32, tag=f"enc{c}",
                                  name=f"psum_enc{c}") for c in range(4)]
                # bias init (start=True clears the bank): out[s, k] = b/8
                for c in range(3):
                    nc.tensor.matmul(psums[c][:], pkc[0:1, 460:460 + B],
                                     brow_t, start=True, stop=False)
                nc.tensor.matmul(psums[3][:], pkc[0:1, 460:460 + B],
                                 brow_t, start=True, stop=False)

                TPC = NT // 4
                ats, vts, wts = [], [], []
                for ch in range(4):
                    at3 = epool.tile([128, TPC * 3 * B], EDT, tag=f"at{ch}",
                                     name=f"at{ch}")
                    nc.gpsimd.dma_start(
                        at3[:], aT[:, ch * TPC * 3 * B:(ch + 1) * TPC * 3 * B])
                    ats.append(at3)
                    vt = epool.tile([128, TPC * B], EDT, tag=f"vt{ch}",
                                    name=f"vt{ch}")
                    nc.gpsimd.dma_start(
                        vt[:], vT[:, ch * TPC * B:(ch + 1) * TPC * B])
                    vts.append(vt)
                    w12t = epool.tile([128, TPC * 2 * DE], EDT, tag=f"wt{ch}",
                                      name=f"wt{ch}")
                    nc.gpsimd.dma_start(
                        w12t[:], w12[:, ch * TPC * 2 * DE:(ch + 1) * TPC * 2 * DE])
                    wts.append(w12t)
                for ch in range(4):
                    for tt in range(TPC):
                        t = ch * TPC + tt
                        last = t == NT - 1
                        at3 = ats[ch][:, tt * 3 * B:(tt + 1) * 3 * B]
                        vt = vts[ch][:, tt * B:(tt + 1) * B]
                        w12t = wts[ch][:, tt * 2 * DE:(tt + 1) * 2 * DE]
                        for c in range(3):
                            nc.tensor.matmul(psums[c][:],
                                             at3[:, c * B:(c + 1) * B], w12t[:],
                                             start=False, stop=last)
                        nc.tensor.matmul(psums[3][:], vt[:], w12t[:],
                                         start=False, stop=last)

                # evict to P: channels use cols 0:DE (W1 half), vis DE:2DE
                for c in range(3):
                    ev = sb.tile([B, DE], F32, tag=f"ev{c}", name=f"ev{c}")
                    nc.any.tensor_copy(ev[:], psums[c][:, 0:DE])
                    nc.gpsimd.dma_start(P[:, c, :], ev[:])
                ev = sb.tile([B, DE], F32, tag="ev3", name="ev3")
                nc.any.tensor_copy(ev[:], psums[3][:, DE:2 * DE])
                nc.gpsimd.dma_start(P[:, 3, :], ev[:])

            # ---------- reduce-scatter: sum over d-shards, scatter samples ----
            nc.gpsimd.collective_compute(
                "ReduceScatter", mybir.AluOpType.add,
                replica_groups=[list(range(NC_))],
                ins=[P[:]], outs=[P_rs[:]],
            )

            # ---------- load this core's 16 samples: [ch, (s, k)] layout ----
            # aud channels and vis kept in separate partition-0-based tiles
            # (compute engines are lane-locked; no partition shifts allowed)
            av_a = sb.tile([3, SK], F32, tag="av_a")   # aud: enc1 + b1
            av_v = sb.tile([3, SK], F32, tag="av_v")   # vis (3 equal rows)
            nc.gpsimd.dma_start(
                av_a[:].rearrange("c (s k) -> c s k", k=DE),
                P_rs[:, 0:3, :].transpose([1, 0, 2]))
            for r in range(3):
                nc.gpsimd.dma_start(
                    av_v[r:r + 1, :].rearrange("c (s k) -> c s k", k=DE),
                    P_rs[:, 3:4, :].transpose([1, 0, 2]))
            av16_a = sb.tile([3, SK], CDT, tag="av16_a")
            nc.any.tensor_copy(av16_a[:], av_a[:])
            av16_v = sb.tile([3, SK], CDT, tag="av16_v")
            nc.any.tensor_copy(av16_v[:], av_v[:])

            with (
                tc.tile_pool(name="att_ps", bufs=3, space="PSUM") as aps,
                tc.tile_pool(name="h_ps", bufs=1, space="PSUM") as hps,
                tc.tile_pool(name="b_ps", bufs=1, space="PSUM") as bps,
                tc.tile_pool(name="o_ps", bufs=1, space="PSUM") as ops_,
            ):
                # ---------- B = A @ av: four K=3 products [3, SK] ----------
                # (Aa|Av) x (aud-half | vis-half) of av
                ba_lo = sb.tile([3, SK], CDT, tag="ba_lo")
                ba_hi = sb.tile([3, SK], CDT, tag="ba_hi")
                bv_lo = sb.tile([3, SK], CDT, tag="bv_lo")
                bv_hi = sb.tile([3, SK], CDT, tag="bv_hi")
                bspec = [(ba_lo, aa_t, av16_a), (ba_hi, aa_t, av16_v),
                         (bv_lo, av_t, av16_a), (bv_hi, av_t, av16_v)]
                for q in range(NQ):
                    ck = slice(q * 512, (q + 1) * 512)
                    for dst, lhs_c, rhs_c in bspec:
                        pb = bps.tile([3, 512], F32, tag="pb")
                        nc.tensor.matmul(pb[:], lhs_c[:], rhs_c[:, ck],
                                         start=True, stop=True)
                        nc.any.tensor_copy(dst[:, ck], pb[:])

                # ---------- attention maps: att = tanh((enc^T @ B) / 16) -------
                att = {
                    (br, half): sb.tile([DE, SK], CDT, tag=f"att_{br}_{half}",
                                        name=f"att_{br}_{half}")
                    for br in ("a", "v") for half in (0, 1)
                }
                blos = {"a": (ba_lo, ba_hi), "v": (bv_lo, bv_hi)}
                enc_rhs = {"a": av16_a, "v": av16_v}
                for q in range(NQ):
                    for br in ("a", "v"):
                        rhs_t = enc_rhs[br]
                        for half in (0, 1):
                            blk = blos[br][half]
                            pa = aps.tile([DE, 512], F32, tag="attp")
                            for j in range(4):
                                s = q * 4 + j
                                sl_ = slice(s * DE, (s + 1) * DE)
                                nc.tensor.matmul(
                                    pa[:, j * DE:(j + 1) * DE],
                                    blk[:, sl_], rhs_t[:, sl_],
                                    start=True, stop=True)
                            nc.scalar.activation(
                                att[(br, half)][:, q * 512:(q + 1) * 512], pa[:],
                                mybir.ActivationFunctionType.Tanh, scale=0.0625)

                # ---------- H = relu(att @ Wc^T + enc^T @ W) ----------
                ht_a = sb.tile([DA, SK], CDT, tag="ht_a")
                ht_v = sb.tile([DA, SK], CDT, tag="ht_v")
                for q in range(NQ):
                    ck = slice(q * 512, (q + 1) * 512)
                    ph_a = hps.tile([DA, 512], F32, tag="ph_a")
                    nc.tensor.matmul(ph_a[:], wa3_t, av16_a[:, ck],
                                     start=True, stop=False)
                    nc.tensor.matmul(ph_a[:], wca_lo, att[("a", 0)][:, ck],
                                     start=False, stop=False)
                    nc.tensor.matmul(ph_a[:], wca_hi, att[("a", 1)][:, ck],
                                     start=False, stop=True)
                    nc.scalar.activation(ht_a[:, ck], ph_a[:],
                                         mybir.ActivationFunctionType.Relu)
                    ph_v = hps.tile([DA, 512], F32, tag="ph_v")
                    nc.tensor.matmul(ph_v[:], wv3_t, av16_v[:, ck],
                                     start=True, stop=False)
                    nc.tensor.matmul(ph_v[:], wcv_lo, att[("v", 0)][:, ck],
                                     start=False, stop=False)
                    nc.tensor.matmul(ph_v[:], wcv_hi, att[("v", 1)][:, ck],
                                     start=False, stop=True)
                    nc.scalar.activation(ht_v[:, ck], ph_v[:],
                                         mybir.ActivationFunctionType.Relu)

                # ---------- out = Wh @ H^T + enc ----------
                outa = sb.tile([3, SK], F32, tag="outa")
                outv = sb.tile([3, SK], F32, tag="outv")
                for q in range(NQ):
                    ck = slice(q * 512, (q + 1) * 512)
                    poa = ops_.tile([3, 512], F32, tag="poa")
                    nc.tensor.matmul(poa[:], wha_t, ht_a[:, ck],
                                     start=True, stop=True)
                    nc.vector.tensor_add(outa[:, ck], poa[:], av_a[:, ck])
                    pov = ops_.tile([3, 512], F32, tag="pov")
                    nc.tensor.matmul(pov[:], whv_t, ht_v[:, ck],
                                     start=True, stop=True)
                    nc.vector.tensor_add(outv[:, ck], pov[:], av_v[:, ck])

            nc.gpsimd.dma_start(
                out[:, :, 0:DE].transpose([1, 0, 2]),
                outa[:].rearrange("c (s k) -> c s k", k=DE))
            nc.gpsimd.dma_start(
                out[:, :, DE:2 * DE].transpose([1, 0, 2]),
                outv[:].rearrange("c (s k) -> c s k", k=DE))

    nc.compile()
    return nc


_NC_CACHE = None


def _get_nc():
    global _NC_CACHE
    if _NC_CACHE is None:
        _NC_CACHE = build_bass()
    return _NC_CACHE


def _prep_inputs(f1_norm, f2_norm, W1, b1, W2, b2, Aa, Av, Wa, Wv,
                 Wca, Wcv, Wha, Whv):
    f1_norm = np.asarray(f1_norm, dtype=np.float32)
    f2_norm = np.asarray(f2_norm, dtype=np.float32)
    edt = _np_dt(EDT)
    cdt = _np_dt(CDT)

    # nearest-downsample + transpose to [d, (c, s)] / [d, s]
    a_ds = f1_norm[:, :, ::4, ::4].reshape(B, 3, D)       # (B, 3, D)
    aT_full = np.ascontiguousarray(a_ds.transpose(2, 1, 0)
                                   .reshape(D, 3 * B)).astype(edt, copy=False)
    v_ds = f2_norm[:, ::4, ::4].reshape(B, D)
    vT_full = np.ascontiguousarray(v_ds.T).astype(edt, copy=False)
    w12_full = np.ascontiguousarray(
        np.concatenate([np.asarray(W1).T, np.asarray(W2).T], axis=1)
    ).astype(edt, copy=False)                              # (D, 256)

    brow = np.concatenate([np.asarray(b1), np.asarray(b2)])[None, :] / NC_
    brow = brow.astype(edt)
    ones = np.ones((1, 3 * B), dtype=edt)

    pk = np.zeros((128, 844), dtype=cdt)
    pk[0:DE, 0:DA] = np.asarray(Wca).T[0:DE]
    pk[0:DE, DA:2 * DA] = np.asarray(Wca).T[DE:2 * DE]
    pk[0:DE, 2 * DA:3 * DA] = np.asarray(Wcv).T[0:DE]
    pk[0:DE, 3 * DA:4 * DA] = np.asarray(Wcv).T[DE:2 * DE]
    pk[0:3, 128:131] = np.asarray(Aa).T
    pk[0:3, 131:134] = np.asarray(Av).T
    pk[0:3, 134:166] = np.asarray(Wa).T
    pk[0:3, 166:198] = np.asarray(Wv).T
    pk[0:DA, 198:201] = np.asarray(Wha).T
    pk[0:DA, 201:204] = np.asarray(Whv).T
    pk[0:1, 204:460] = brow.astype(cdt)
    pk[0:1, 460:844] = ones.astype(cdt)
    consts = {"pk16": pk}

    def tile128(arr, ncols):
        # [DL, ncols] -> [128, NT*ncols]: row p holds k-tiles t at col t*ncols
        return np.ascontiguousarray(
            arr.reshape(NT, 128, ncols).transpose(1, 0, 2)
            .reshape(128, NT * ncols))

    in_maps = []
    for i in range(NC_):
        rs = slice(i * DL, (i + 1) * DL)
        m = {
            "aT": tile128(aT_full[rs], 3 * B),
            "vT": tile128(vT_full[rs], B),
            "w12": tile128(w12_full[rs], 2 * DE),
        }
        m.update(consts)
        in_maps.append(m)
    return in_maps


def _run(inputs, trace=False):
    nc = _get_nc()
    in_maps = _prep_inputs(**inputs)
    res = run_bass_kernel_spmd(nc, in_maps, list(range(NC_)), trace=trace)
    full = np.concatenate([res.results[i]["out"] for i in range(NC_)], axis=0)
    return full.astype(np.float32, copy=False), res


def kernel(**inputs):
    out, _ = _run(inputs, trace=False)
    return out

